# revision 19
# baseline (speedup 1.0000x reference)
"""Trainium2 Bass kernel for nn_BitSpikeMambaModel (embed -> bitlinear x2 -> LN -> bitlinear head).

Self-contained: hardcodes shapes from the problem spec.

Sharding:
  - trunk (embed, L0, L1, LN): data-parallel over the 4096 tokens
    (512 tokens per core, 8 cores), weights replicated.
  - head bitlinear: tensor-parallel over the vocab dim. Each core owns a
    ~4000-row slice of head_w and computes logits for ALL 4096 tokens
    against its slice.

Key scheduling ideas (engine queues are strict FIFO and the FIRST
collective absorbs the multi-core launch stagger, so the schedule keeps
collectives off every critical path):
  - NO AllReduce for the w0/w1 BitNet scales: each core abs-sums fp16
    copies of the full w0/w1 on DVE (the fp16 |w|-sum moves the threshold
    by ~1e-7 relative -> ~0 flipped quant decisions). The embedding
    gather is the first gpsimd op; L0 starts ~40us in.
  - head abs-sum chunks (fp16 copy) stream from t=0; AllReduce #2 fires
    mid-L0 and its ring + core-stagger hide inside the L0/L1 window.
  - LN activations AllGather in 2 UNEVEN token slices (64 + 448 local
    tokens), written to separate SBUF tiles so the AG input DMAs are
    contiguous. The head processes slice 0 as soon as the small AG
    lands, hiding the big AG behind ~134us of matmul work; acts loads
    ride the gpsimd queue interleaved with the collectives.
  - BitNet ternary quantization on device, fused to 2 DVE ops producing
    -q: qneg = 1{w<-h} - 1{w>h} with h = 0.5*scale; PSUM evictions use
    scale=-s to restore the sign. Weight THRESHOLD COMPARES use fp32
    streams (fp16-rounded weights flip ~4e-5 of decisions -> 3e-2 err).
  - all matmuls fp16 (ternary weights exact in fp16), fp32 PSUM accum,
    512-col moving data (PSUM bank limit). (fp8 DoubleRow simulated on
    host: e4m3 activations alone give 2.5e-2 absmax-rel vs the 2e-2
    gate -- not usable.)

Output per core: out0 [4096, 8*64] + out1 [4096, 8*448] fp16
(vocab-slice rows x (rank, slice-token)); host stitches to [2,S,V] f32.
"""

import numpy as np

import concourse.bass as bass
import concourse.bacc as bacc
import concourse.mybir as mybir
import concourse.tile as tile
from concourse.bass_utils import run_bass_kernel_spmd

F32 = mybir.dt.float32
F16 = mybir.dt.float16
I16 = mybir.dt.int16
AF = mybir.ActivationFunctionType
OP = mybir.AluOpType
AX = mybir.AxisListType

VOCAB = 32000
DIM = 2048
BATCH = 2
SEQ = 2048
NCORES = 8
EPS = 1e-5

T = (BATCH * SEQ) // NCORES       # 512 local tokens per core
TF = BATCH * SEQ                  # 4096 total tokens
DT = DIM // 128                   # 16 d-tiles
NV = VOCAB // 128                 # 250 head o-tiles total
HT = 32                           # head o-tiles per core (padded)
WC = DIM // 256                   # 8 abs-sum chunks for w0/w1 (fp16)
HC = 16                           # head abs-sum chunks (256 cols each)
TBS = [64, 448]                   # token-slice AllGather sizes
NMM = [(NCORES * tb) // T for tb in TBS]   # 512-col matmuls per dt: 1, 7

# per-core head tile assignment: cores 0,1 get 32 tiles, cores 2..7 get 31
_CNT = [32, 32] + [31] * 6
_START = np.concatenate([[0], np.cumsum(_CNT)[:-1]]).tolist()
assert sum(_CNT) == NV


class Cfg:
    def __init__(self, G=2):
        self.G = G                # o-tile group size for trunk weight streaming


def build(cfg: Cfg):
    G = cfg.G
    nc = bacc.Bacc("TRN2", target_bir_lowering=False, debug=False,
                   num_devices=NCORES)

    # ---- DRAM I/O ----
    idx_d = nc.dram_tensor("idx", [128, T // 16], I16, kind="ExternalInput")
    embh_d = nc.dram_tensor("embh", [VOCAB, DIM], F16, kind="ExternalInput")
    w0t_d = nc.dram_tensor("w0t", [DIM, DIM], F32, kind="ExternalInput")
    w1t_d = nc.dram_tensor("w1t", [DIM, DIM], F32, kind="ExternalInput")
    w0h_d = nc.dram_tensor("w0h", [DIM, DIM], F16, kind="ExternalInput")
    w1h_d = nc.dram_tensor("w1h", [DIM, DIM], F16, kind="ExternalInput")
    wslh_d = nc.dram_tensor("wslh", [DIM, HT * 128], F32, kind="ExternalInput")
    wslh16_d = nc.dram_tensor("wslh16", [DIM, HT * 128], F16, kind="ExternalInput")
    b0_d = nc.dram_tensor("b0r", [128, DT], F32, kind="ExternalInput")
    b1_d = nc.dram_tensor("b1r", [128, DT], F32, kind="ExternalInput")
    gam_d = nc.dram_tensor("gamr", [128, DT], F32, kind="ExternalInput")
    bet_d = nc.dram_tensor("betr", [128, DT], F32, kind="ExternalInput")
    hb_d = nc.dram_tensor("hbr", [128, HT], F32, kind="ExternalInput")
    out_b = [nc.dram_tensor(f"out{i}", [HT * 128, NCORES * TBS[i]], F16,
                            kind="ExternalOutput") for i in range(len(TBS))]

    w0t_v = w0t_d.ap().rearrange("(dt p) o -> p dt o", p=128)
    w1t_v = w1t_d.ap().rearrange("(dt p) o -> p dt o", p=128)
    w0h_v = w0h_d.ap().rearrange("(dt p) o -> p dt o", p=128)
    w1h_v = w1h_d.ap().rearrange("(dt p) o -> p dt o", p=128)
    wslh_v = wslh_d.ap().rearrange("(dt p) o -> p dt o", p=128)
    wslh16_v = wslh16_d.ap().rearrange("(dt p) o -> p dt o", p=128)
    groups = [list(range(NCORES))]

    import contextlib
    with tile.TileContext(nc) as tc:
        with contextlib.ExitStack() as ctx:
            cst = ctx.enter_context(tc.tile_pool(name="cst", bufs=1))
            sml = ctx.enter_context(tc.tile_pool(name="sml", bufs=1))
            scl = ctx.enter_context(tc.tile_pool(name="scl", bufs=1))
            drp = ctx.enter_context(tc.tile_pool(name="drp", bufs=1, space="DRAM"))

            # collective buffers (internal DRAM; outputs Shared for HBM-HBM path)
            ar2_in = drp.tile([1, 1], F32, tag="ar2i")
            ar2_out = drp.tile([1, 1], F32, tag="ar2o", addr_space="Shared")
            ag_in_q = [drp.tile([128, DT * TBS[i]], F16, tag=f"agi{i}",
                                name=f"agi{i}") for i in range(len(TBS))]
            ag_out_q = [drp.tile([NCORES, 128, DT * TBS[i]], F16, tag=f"ago{i}",
                                 name=f"ago{i}", addr_space="Shared")
                        for i in range(len(TBS))]

            # ---- constants ----
            ones_col = cst.tile([128, 1], F16)
            nc.any.memset(ones_col[:], 1.0)
            ones_colf = cst.tile([128, 1], F32)
            nc.any.memset(ones_colf[:], 1.0)
            ones_row = cst.tile([1, 128], F32)
            nc.any.memset(ones_row[:], 1.0)
            eps1 = cst.tile([1, 1], F32)
            nc.any.memset(eps1[:], EPS)
            idx_sb = cst.tile([128, T // 16], I16)
            nc.sync.dma_start(idx_sb[:], idx_d.ap())
            b0s = cst.tile([128, DT], F32)
            nc.scalar.dma_start(b0s[:], b0_d.ap())
            b1s = cst.tile([128, DT], F32)
            nc.scalar.dma_start(b1s[:], b1_d.ap())
            gams = cst.tile([128, DT], F32)
            nc.scalar.dma_start(gams[:], gam_d.ap())
            bets = cst.tile([128, DT], F32)
            nc.scalar.dma_start(bets[:], bet_d.ap())
            hbs = cst.tile([128, HT], F32)
            nc.scalar.dma_start(hbs[:], hb_d.ap())

            with contextlib.ExitStack() as trunk_ctx:
                big = trunk_ctx.enter_context(tc.tile_pool(name="big", bufs=1))
                evt = trunk_ctx.enter_context(tc.tile_pool(name="evt", bufs=2))
                wstream = trunk_ctx.enter_context(tc.tile_pool(name="wstream", bufs=3))
                hch = trunk_ctx.enter_context(tc.tile_pool(name="hch", bufs=3))
                qbuf = trunk_ctx.enter_context(tc.tile_pool(name="qbuf", bufs=2))
                ps_mm = trunk_ctx.enter_context(
                    tc.tile_pool(name="ps_mm", bufs=3, space="PSUM"))
                ps_st = trunk_ctx.enter_context(
                    tc.tile_pool(name="ps_st", bufs=1, space="PSUM"))

                # ---- embedding gather: FIRST gpsimd op, no collective ahead
                xt = big.tile([128, DT, T], F16, tag="xt")
                nc.gpsimd.dma_gather(out_ap=xt[:], in_ap=embh_d.ap(),
                                     idxs_ap=idx_sb[:], num_idxs=T,
                                     num_idxs_reg=T, elem_size=DIM,
                                     transpose=True)

                # [128,1] partial sums -> [1,1] total via ones-matmul
                def sum_tail(p3, tagsuf):
                    tps = ps_st.tile([1, 1], F32, tag="pa")
                    nc.tensor.matmul(tps[:], ones_colf[:], p3[:], start=True,
                                     stop=True)
                    tot = sml.tile([1, 1], F32, tag=f"tot{tagsuf}")
                    nc.scalar.activation(tot[:], tps[:], AF.Copy)
                    return tot

                # [1,1] total -> sneg=-max(mean,EPS) [128,1], h=s/2, -h
                def finalize_scale(tot_sb, mul, tagsuf):
                    rps = ps_st.tile([128, 1], F32, tag="pa")
                    nc.tensor.matmul(rps[:], ones_row[:], tot_sb[:], start=True,
                                     stop=True)
                    s = scl.tile([128, 1], F32, tag=f"s{tagsuf}")
                    nc.scalar.activation(s[:], rps[:], AF.Copy, scale=mul)
                    nc.vector.tensor_scalar(s[:], s[:], EPS, None, OP.max)
                    sneg = scl.tile([128, 1], F32, tag=f"sn{tagsuf}")
                    nc.vector.tensor_scalar(sneg[:], s[:], -1.0, None, OP.mult)
                    h = scl.tile([128, 1], F32, tag=f"h{tagsuf}")
                    nc.vector.tensor_scalar(h[:], s[:], 0.5, None, OP.mult)
                    nh = scl.tile([128, 1], F32, tag=f"nh{tagsuf}")
                    nc.vector.tensor_scalar(nh[:], h[:], -1.0, None, OP.mult)
                    return sneg, h, nh

                # fp16 [DIM, 256]-chunk abs-sum into an accumulator on DVE
                accs = {}

                def abs_chunk(view, ch, acc):
                    wt = hch.tile([128, DT, 256], F16, tag="hch")
                    nc.scalar.dma_start(wt[:], view[:, :, ch * 256:(ch + 1) * 256])
                    part = sml.tile([128, DT, 2], F32, tag="partc")
                    nc.vector.tensor_reduce(
                        part[:], wt[:].rearrange("p dt (c k) -> p dt c k", k=128),
                        axis=AX.X, op=OP.add, apply_absolute_value=True)
                    p3 = sml.tile([128, 1], F32, tag="p3c")
                    nc.vector.tensor_reduce(
                        p3[:], part[:].rearrange("p dt c -> p (dt c)"),
                        axis=AX.X, op=OP.add)
                    if acc not in accs:
                        accs[acc] = sml.tile([128, 1], F32, tag=acc, name=acc)
                        nc.vector.tensor_copy(accs[acc][:], p3[:])
                    else:
                        nc.vector.tensor_tensor(accs[acc][:], accs[acc][:],
                                                p3[:], OP.add)

                # w0 scale: local fp16 abs-sum (no collective)
                for ch in range(WC):
                    abs_chunk(w0h_v, ch, "w0acc")
                sn0, h0, nh0 = finalize_scale(sum_tail(accs["w0acc"], "w0"),
                                              1.0 / (DIM * DIM), "w0")

                # ---- streamed bitlinear for trunk ----
                def bitlinear(wview, h_ap, nh_ap, rhs, consume, chunks):
                    for g in range(DT // G):
                        wt = wstream.tile([128, DT, G * 128], F32, tag="wstream")
                        nc.sync.dma_start(wt[:], wview[:, :, g * G * 128:(g + 1) * G * 128])
                        sgn = qbuf.tile([128, DT, G * 128], F16, tag="sgn")
                        nc.vector.tensor_scalar(sgn[:], wt[:], h_ap[:], None, OP.is_gt)
                        qng = qbuf.tile([128, DT, G * 128], F16, tag="qng")
                        nc.vector.scalar_tensor_tensor(qng[:], wt[:], nh_ap[:], sgn[:],
                                                       OP.is_lt, OP.subtract)
                        for fn in chunks.get(g, []):
                            fn()
                        for j in range(G):
                            ot = g * G + j
                            pt = ps_mm.tile([128, T], F32, tag="ps_mm")
                            for dt in range(DT):
                                nc.tensor.matmul(pt[:], qng[:, dt, j * 128:(j + 1) * 128],
                                                 rhs[:, dt, :],
                                                 start=(dt == 0), stop=(dt == DT - 1))
                            consume(ot, pt)

                # ---- layer 0; head chunks + w1 chunks + AR2 interleaved ----
                h1sb = big.tile([128, DT, T], F16, tag="h1sb")

                def consume_l0(ot, pt):
                    nc.scalar.activation(h1sb[:, ot, :], pt[:], AF.Identity,
                                         bias=b0s[:, ot:ot + 1], scale=sn0[:])

                toth_g = sml.tile([1, 1], F32, tag="tothg")

                def fire_ar2():
                    toth = sum_tail(accs["hacc"], "hd")
                    nc.scalar.dma_start(ar2_in[:], toth[:])
                    nc.gpsimd.collective_compute(
                        "AllReduce", OP.add, replica_groups=groups,
                        ins=[ar2_in[:]], outs=[ar2_out[:]])
                    nc.scalar.dma_start(toth_g[:], ar2_out[:])

                # head chunks during L0 groups 0..5 (16 chunks), AR2 after g5,
                # w1 fp16 chunks during g6..7 (6 of 8)
                l0_sched = {g: [lambda c=c: abs_chunk(wslh16_v, c, "hacc")
                                for c in range(3 * g, min(3 * g + 3, HC))]
                            for g in range(6)}
                l0_sched[5].append(fire_ar2)
                for g in (6, 7):
                    l0_sched[g] = [lambda c=c: abs_chunk(w1h_v, c, "w1acc")
                                   for c in range(3 * (g - 6), 3 * (g - 6) + 3)]

                bitlinear(w0t_v, h0, nh0, xt, consume_l0, l0_sched)

                # finish w1 abs-sum (last 2 chunks) then finalize its scale
                for c in (6, 7):
                    abs_chunk(w1h_v, c, "w1acc")
                sn1, h1, nh1 = finalize_scale(sum_tail(accs["w1acc"], "w1"),
                                              1.0 / (DIM * DIM), "w1")

                # ---- layer 1 + LN stats ----
                h2sb = big.tile([128, DT, T], F16, tag="h2sb")
                ps_s = ps_st.tile([1, T], F32, tag="ps_s")
                ps_q = ps_st.tile([1, T], F32, tag="ps_q")

                def consume_l1(ot, pt):
                    nc.scalar.activation(h2sb[:, ot, :], pt[:], AF.Identity,
                                         bias=b1s[:, ot:ot + 1], scale=sn1[:])
                    sq = evt.tile([128, T], F16, tag="evt")
                    nc.scalar.activation(sq[:], pt[:], AF.Square,
                                         bias=b1s[:, ot:ot + 1], scale=sn1[:])
                    nc.tensor.matmul(ps_s[:], ones_col[:], h2sb[:, ot, :],
                                     start=(ot == 0), stop=(ot == DT - 1))
                    nc.tensor.matmul(ps_q[:], ones_col[:], sq[:],
                                     start=(ot == 0), stop=(ot == DT - 1))

                bitlinear(w1t_v, h1, nh1, h1sb, consume_l1, {})

                # head scale finalize: PE ops sit after L1 so the PE FIFO
                # doesn't stall on AR2
                snh, hh, nhh = finalize_scale(toth_g, 1.0 / (DIM * VOCAB), "hd")

                # ---- layernorm ----
                mu = sml.tile([1, T], F32, tag="mu")
                nc.scalar.activation(mu[:], ps_s[:], AF.Copy, scale=1.0 / DIM)
                ms = sml.tile([1, T], F32, tag="ms")
                nc.scalar.activation(ms[:], ps_q[:], AF.Copy, scale=1.0 / DIM)
                var = sml.tile([1, T], F32, tag="var")
                nc.vector.tensor_tensor(var[:], mu[:], mu[:], OP.mult)
                nc.vector.tensor_tensor(var[:], ms[:], var[:], OP.subtract)
                sd = sml.tile([1, T], F32, tag="sd")
                nc.scalar.activation(sd[:], var[:], AF.Sqrt, bias=eps1[:])
                rstd = sml.tile([1, T], F32, tag="rstd")
                nc.vector.reciprocal(rstd[:], sd[:])
                negmur = sml.tile([1, T], F32, tag="r0")
                nc.vector.tensor_tensor(negmur[:], mu[:], rstd[:], OP.mult)
                nc.vector.tensor_scalar(negmur[:], negmur[:], -1.0, None, OP.mult)
                # broadcast to [128, T] via ones-matmul
                pa = ps_st.tile([128, T], F32, tag="pa")
                nc.tensor.matmul(pa[:], ones_row[:], rstd[:], start=True, stop=True)
                a_b = big.tile([128, T], F32)
                nc.scalar.activation(a_b[:], pa[:], AF.Copy)
                pb = ps_st.tile([128, T], F32, tag="pa")
                nc.tensor.matmul(pb[:], ones_row[:], negmur[:], start=True, stop=True)
                b_b = big.tile([128, T], F32)
                nc.scalar.activation(b_b[:], pb[:], AF.Copy)

                # apply LN -> fp16 into per-slice tiles (contiguous AG inputs)
                xs = [big.tile([128, DT, tb], F16, tag=f"xs{i}", name=f"xs{i}")
                      for i, tb in enumerate(TBS)]
                for dt in range(DT):
                    t1 = evt.tile([128, T], F32, tag="evtf")
                    nc.vector.tensor_tensor(t1[:], h2sb[:, dt, :], a_b[:], OP.mult)
                    nc.vector.tensor_tensor(t1[:], t1[:], b_b[:], OP.add)
                    off = 0
                    for i, tb in enumerate(TBS):
                        nc.scalar.activation(xs[i][:, dt, :], t1[:, off:off + tb],
                                             AF.Identity,
                                             bias=bets[:, dt:dt + 1],
                                             scale=gams[:, dt:dt + 1])
                        off += tb

                for i, tb in enumerate(TBS):
                    nc.sync.dma_start(
                        ag_in_q[i][:].rearrange("p (dt t) -> p dt t", t=tb),
                        xs[i][:])

            # ---- head phase: token-slice-major ----
            with contextlib.ExitStack() as head_ctx:
                acts_p = head_ctx.enter_context(tc.tile_pool(name="acts", bufs=1))
                hws = head_ctx.enter_context(tc.tile_pool(name="hws", bufs=4))
                hqb = head_ctx.enter_context(tc.tile_pool(name="hqb", bufs=2))
                osb = head_ctx.enter_context(tc.tile_pool(name="osb", bufs=2))

                # collectives + acts loads interleaved on the gpsimd queue:
                # acts_b[i] is ready right after AG i, and nothing later on
                # the gpsimd queue is needed early.
                acts_b = []
                for i, tb in enumerate(TBS):
                    ab = acts_p.tile([128, DT, NCORES, tb], F16, tag=f"actb{i}",
                                     name=f"actb{i}")
                    nc.gpsimd.collective_compute(
                        "AllGather", OP.bypass, replica_groups=groups,
                        ins=[ag_in_q[i][:]], outs=[ag_out_q[i][:]])
                    for r in range(NCORES):
                        nc.gpsimd.dma_start(
                            ab[:, :, r, :], ag_out_q[i][:][r].rearrange(
                                "p (dt t) -> p dt t", t=tb))
                    acts_b.append(ab)

                def quant_head(g):
                    wt = hws.tile([128, DT, 128], F32, tag="hws")
                    nc.sync.dma_start(wt[:], wslh_v[:, :, g * 128:(g + 1) * 128])
                    sgn = hqb.tile([128, DT, 128], F16, tag="sgn")
                    nc.vector.tensor_scalar(sgn[:], wt[:], hh[:], None, OP.is_gt)
                    qng = hqb.tile([128, DT, 128], F16, tag="qng")
                    nc.vector.scalar_tensor_tensor(qng[:], wt[:], nhh[:], sgn[:],
                                                   OP.is_lt, OP.subtract)
                    return qng

                def head_mm(ps_h, b, g, qng):
                    # NMM[b] 512-col matmuls per dt sharing one lhsT load
                    n = NMM[b]
                    rv = acts_b[b][:].rearrange("p dt r t -> p dt (r t)")
                    pts = [ps_h.tile([128, T], F32, tag=f"ps{b}_{k}",
                                     name=f"ps{b}_{k}") for k in range(n)]
                    for dt in range(DT):
                        for k in range(n):
                            nc.tensor.matmul(pts[k][:], qng[:, dt, :],
                                             rv[:, dt, k * T:(k + 1) * T],
                                             start=(dt == 0), stop=(dt == DT - 1))
                    o = osb.tile([128, NCORES * TBS[b]], F16, tag=f"osb{b}")
                    for k in range(n):
                        nc.scalar.activation(o[:, k * T:(k + 1) * T], pts[k][:],
                                             AF.Identity, bias=hbs[:, g:g + 1],
                                             scale=snh[:])
                    nc.scalar.dma_start(
                        out_b[b].ap()[g * 128:(g + 1) * 128, :], o[:])

                # pass 1: small slice (starts as soon as AG 0 lands)
                with tc.tile_pool(name="ps_h1", bufs=3, space="PSUM") as ps_h1:
                    for g in range(HT):
                        head_mm(ps_h1, 0, g, quant_head(g))

                # pass 2: big slice
                with tc.tile_pool(name="ps_h2", bufs=1, space="PSUM") as ps_h2:
                    for g in range(HT):
                        head_mm(ps_h2, 1, g, quant_head(g))

    nc.compile()
    return nc


_BUILD_CACHE = {}


def _get_nc(cfg: Cfg):
    key = (cfg.G,)
    if key not in _BUILD_CACHE:
        _BUILD_CACHE[key] = build(cfg)
    return _BUILD_CACHE[key]


def make_in_maps(cfg: Cfg, x, emb, w0, b0, w1, b1, ln_gamma, ln_beta, head_w, head_b):
    """Host-side sharding/layout prep. Returns list of per-core input dicts."""
    embh = np.asarray(emb, np.float32).astype(np.float16)
    w0t = np.ascontiguousarray(np.asarray(w0, np.float32).T)
    w1t = np.ascontiguousarray(np.asarray(w1, np.float32).T)
    hwt = np.ascontiguousarray(np.asarray(head_w, np.float32).T)  # [D, V]

    def rearr(v, n):
        return np.ascontiguousarray(np.asarray(v, np.float32).reshape(n, 128).T)

    b0r = rearr(b0, DT)
    b1r = rearr(b1, DT)
    gamr = rearr(ln_gamma, DT)
    betr = rearr(ln_beta, DT)
    hb = np.asarray(head_b, np.float32)

    ids = np.asarray(x).reshape(-1).astype(np.int16)
    assert ids.size == NCORES * T
    in_maps = []
    for c in range(NCORES):
        # indices wrapped into 16 partitions, replicated across the 8 Q7 stripes
        idx_arr = np.tile(ids[c * T:(c + 1) * T].reshape(T // 16, 16).T, (8, 1))
        lo, cnt = _START[c] * 128, _CNT[c] * 128
        wslh = np.zeros((DIM, HT * 128), np.float32)
        wslh[:, :cnt] = hwt[:, lo:lo + cnt]
        hbr = np.zeros((HT * 128,), np.float32)
        hbr[:cnt] = hb[lo:lo + cnt]
        in_maps.append(dict(
            idx=idx_arr, embh=embh, w0t=w0t, w1t=w1t,
            w0h=w0t.astype(np.float16), w1h=w1t.astype(np.float16),
            wslh=wslh, wslh16=wslh.astype(np.float16),
            b0r=b0r, b1r=b1r, gamr=gamr, betr=betr,
            hbr=rearr(hbr, HT)))
    return in_maps


def _run(cfg: Cfg, inputs, trace=False):
    nc = _get_nc(cfg)
    in_maps = make_in_maps(cfg, **inputs)
    res = run_bass_kernel_spmd(nc, in_maps, core_ids=list(range(NCORES)),
                               trace=trace)
    full = np.empty((TF, VOCAB), np.float32)
    offs = np.concatenate([[0], np.cumsum(TBS)[:-1]])
    for c in range(NCORES):
        lo, cnt = _START[c] * 128, _CNT[c] * 128
        for i, tb in enumerate(TBS):
            # out_b[i]: rows = o, cols = (rank, slice-token)
            o = res.results[c][f"out{i}"].reshape(HT * 128, NCORES, tb)
            for r in range(NCORES):
                t0 = r * T + offs[i]
                full[t0:t0 + tb, lo:lo + cnt] = o[:cnt, r, :].T
    return full, res


def kernel(**inputs) -> np.ndarray:
    cfg = Cfg()
    full, _ = _run(cfg, inputs)
    return full.reshape(BATCH, SEQ, VOCAB)


# revision 23
# speedup vs baseline: 1.0045x; 1.0045x over previous
"""Trainium2 Bass kernel for nn_BitSpikeMambaModel (embed -> bitlinear x2 -> LN -> bitlinear head).

Self-contained: hardcodes shapes from the problem spec.

Sharding:
  - trunk (embed, L0, L1, LN): data-parallel over the 4096 tokens
    (512 tokens per core, 8 cores), weights replicated.
  - head bitlinear: tensor-parallel over the vocab dim. Each core owns a
    ~4000-row slice of head_w and computes logits for ALL 4096 tokens
    against its slice.

Key scheduling ideas (engine queues are strict FIFO and the FIRST
collective absorbs the multi-core launch stagger, so the schedule keeps
collectives off every critical path):
  - NO AllReduce for the w0/w1 BitNet scales: each core abs-sums fp16
    copies of the full w0/w1 on DVE (the fp16 |w|-sum moves the threshold
    by ~1e-7 relative -> ~0 flipped quant decisions). The embedding
    gather is the first gpsimd op; L0 starts ~40us in.
  - head abs-sum chunks (fp16 copy) stream from t=0; AllReduce #2 fires
    mid-L0 and its ring + core-stagger hide inside the L0/L1 window.
  - LN activations AllGather in 2 UNEVEN token slices (64 + 448 local
    tokens), written to separate SBUF tiles so the AG input DMAs are
    contiguous. The head processes slice 0 as soon as the small AG
    lands, hiding the big AG behind ~134us of matmul work; acts loads
    ride the gpsimd queue interleaved with the collectives.
  - BitNet ternary quantization on device, fused to 2 DVE ops producing
    -q: qneg = 1{w<-h} - 1{w>h} with h = 0.5*scale; PSUM evictions use
    scale=-s to restore the sign. Weight THRESHOLD COMPARES use fp32
    streams (fp16-rounded weights flip ~4e-5 of decisions -> 3e-2 err).
  - all matmuls fp16 (ternary weights exact in fp16), fp32 PSUM accum,
    512-col moving data (PSUM bank limit). (fp8 DoubleRow simulated on
    host: e4m3 activations alone give 2.5e-2 absmax-rel vs the 2e-2
    gate -- not usable.)

Output per core: out0 [4096, 8*64] + out1 [4096, 8*448] fp16
(vocab-slice rows x (rank, slice-token)); host stitches to [2,S,V] f32.
"""

import numpy as np

import concourse.bass as bass
import concourse.bacc as bacc
import concourse.mybir as mybir
import concourse.tile as tile
from concourse.bass_utils import run_bass_kernel_spmd

F32 = mybir.dt.float32
F16 = mybir.dt.float16
I16 = mybir.dt.int16
AF = mybir.ActivationFunctionType
OP = mybir.AluOpType
AX = mybir.AxisListType

VOCAB = 32000
DIM = 2048
BATCH = 2
SEQ = 2048
NCORES = 8
EPS = 1e-5

T = (BATCH * SEQ) // NCORES       # 512 local tokens per core
TF = BATCH * SEQ                  # 4096 total tokens
DT = DIM // 128                   # 16 d-tiles
NV = VOCAB // 128                 # 250 head o-tiles total
HT = 32                           # head o-tiles per core (padded)
WC = DIM // 256                   # 8 abs-sum chunks for w0/w1 (fp16)
HC = 16                           # head abs-sum chunks (256 cols each)
TBS = [64, 448]                   # token-slice AllGather sizes
NMM = [(NCORES * tb) // T for tb in TBS]   # 512-col matmuls per dt: 1, 7

# per-core head tile assignment: cores 0,1 get 32 tiles, cores 2..7 get 31
_CNT = [32, 32] + [31] * 6
_START = np.concatenate([[0], np.cumsum(_CNT)[:-1]]).tolist()
assert sum(_CNT) == NV


class Cfg:
    def __init__(self, G=2):
        self.G = G                # o-tile group size for trunk weight streaming


def build(cfg: Cfg):
    G = cfg.G
    nc = bacc.Bacc("TRN2", target_bir_lowering=False, debug=False,
                   num_devices=NCORES)

    # ---- DRAM I/O ----
    idx_d = nc.dram_tensor("idx", [128, T // 16], I16, kind="ExternalInput")
    embh_d = nc.dram_tensor("embh", [VOCAB, DIM], F16, kind="ExternalInput")
    w0t_d = nc.dram_tensor("w0t", [DIM, DIM], F32, kind="ExternalInput")
    w1t_d = nc.dram_tensor("w1t", [DIM, DIM], F32, kind="ExternalInput")
    w0h_d = nc.dram_tensor("w0h", [DIM, DIM], F16, kind="ExternalInput")
    w1h_d = nc.dram_tensor("w1h", [DIM, DIM], F16, kind="ExternalInput")
    wslh_d = nc.dram_tensor("wslh", [DIM, HT * 128], F32, kind="ExternalInput")
    wslh16_d = nc.dram_tensor("wslh16", [DIM, HT * 128], F16, kind="ExternalInput")
    b0_d = nc.dram_tensor("b0r", [128, DT], F32, kind="ExternalInput")
    b1_d = nc.dram_tensor("b1r", [128, DT], F32, kind="ExternalInput")
    gam_d = nc.dram_tensor("gamr", [128, DT], F32, kind="ExternalInput")
    bet_d = nc.dram_tensor("betr", [128, DT], F32, kind="ExternalInput")
    hb_d = nc.dram_tensor("hbr", [128, HT], F32, kind="ExternalInput")
    eye_d = nc.dram_tensor("eye16", [128, 128], F16, kind="ExternalInput")
    out_b = [nc.dram_tensor(f"out{i}", [HT * 128, NCORES * TBS[i]], F16,
                            kind="ExternalOutput") for i in range(len(TBS))]

    w0t_v = w0t_d.ap().rearrange("(dt p) o -> p dt o", p=128)
    w1t_v = w1t_d.ap().rearrange("(dt p) o -> p dt o", p=128)
    w0h_v = w0h_d.ap().rearrange("(dt p) o -> p dt o", p=128)
    w1h_v = w1h_d.ap().rearrange("(dt p) o -> p dt o", p=128)
    wslh_v = wslh_d.ap().rearrange("(dt p) o -> p dt o", p=128)
    wslh16_v = wslh16_d.ap().rearrange("(dt p) o -> p dt o", p=128)
    groups = [list(range(NCORES))]

    import contextlib
    with tile.TileContext(nc) as tc:
        with contextlib.ExitStack() as ctx:
            cst = ctx.enter_context(tc.tile_pool(name="cst", bufs=1))
            sml = ctx.enter_context(tc.tile_pool(name="sml", bufs=1))
            scl = ctx.enter_context(tc.tile_pool(name="scl", bufs=1))
            drp = ctx.enter_context(tc.tile_pool(name="drp", bufs=1, space="DRAM"))

            # collective buffers (internal DRAM; outputs Shared for HBM-HBM path)
            ar2_in = drp.tile([1, 1], F32, tag="ar2i")
            ar2_out = drp.tile([1, 1], F32, tag="ar2o", addr_space="Shared")
            ag_in_q = [drp.tile([128, DT * TBS[i]], F16, tag=f"agi{i}",
                                name=f"agi{i}") for i in range(len(TBS))]
            ag_out_q = [drp.tile([NCORES, 128, DT * TBS[i]], F16, tag=f"ago{i}",
                                 name=f"ago{i}", addr_space="Shared")
                        for i in range(len(TBS))]

            # ---- constants ----
            ones_col = cst.tile([128, 1], F16)
            nc.any.memset(ones_col[:], 1.0)
            ones_colf = cst.tile([128, 1], F32)
            nc.any.memset(ones_colf[:], 1.0)
            ones_row = cst.tile([1, 128], F32)
            nc.any.memset(ones_row[:], 1.0)
            eps1 = cst.tile([1, 1], F32)
            nc.any.memset(eps1[:], EPS)
            idx_sb = cst.tile([128, T // 16], I16)
            nc.sync.dma_start(idx_sb[:], idx_d.ap())
            b0s = cst.tile([128, DT], F32)
            nc.scalar.dma_start(b0s[:], b0_d.ap())
            b1s = cst.tile([128, DT], F32)
            nc.scalar.dma_start(b1s[:], b1_d.ap())
            gams = cst.tile([128, DT], F32)
            nc.scalar.dma_start(gams[:], gam_d.ap())
            bets = cst.tile([128, DT], F32)
            nc.scalar.dma_start(bets[:], bet_d.ap())
            hbs = cst.tile([128, HT], F32)
            nc.scalar.dma_start(hbs[:], hb_d.ap())
            eye16 = cst.tile([128, 128], F16)
            nc.sync.dma_start(eye16[:], eye_d.ap())

            with contextlib.ExitStack() as trunk_ctx:
                big = trunk_ctx.enter_context(tc.tile_pool(name="big", bufs=1))
                evt = trunk_ctx.enter_context(tc.tile_pool(name="evt", bufs=2))
                wstream = trunk_ctx.enter_context(tc.tile_pool(name="wstream", bufs=2))
                hch = trunk_ctx.enter_context(tc.tile_pool(name="hch", bufs=3))
                hch2 = trunk_ctx.enter_context(tc.tile_pool(name="hch2", bufs=2))
                qbuf = trunk_ctx.enter_context(tc.tile_pool(name="qbuf", bufs=2))
                ps_mm = trunk_ctx.enter_context(
                    tc.tile_pool(name="ps_mm", bufs=3, space="PSUM"))
                ps_st = trunk_ctx.enter_context(
                    tc.tile_pool(name="ps_st", bufs=1, space="PSUM"))

                # ---- embedding gather: FIRST gpsimd op, no collective
                # ahead. Non-transpose mode (contiguous 4KB row writes; the
                # transpose-mode 2-byte scatter took 40-60us), then PE
                # transposes the 64 [tok,d] tiles via identity matmuls.
                xt_raw = big.tile([128, T // 128, DIM], F16, tag="xtr")
                nc.gpsimd.dma_gather(out_ap=xt_raw[:], in_ap=embh_d.ap(),
                                     idxs_ap=idx_sb[:], num_idxs=T,
                                     num_idxs_reg=T, elem_size=DIM,
                                     transpose=False)
                xt = big.tile([128, DT, T], F16, tag="xt")
                for tb in range(T // 128):
                    for dt in range(DT):
                        ptr = ps_st.tile([128, 128], F16, tag=f"ptr{dt % 2}",
                                         name="ptr")
                        nc.tensor.matmul(ptr[:],
                                         xt_raw[:, tb, dt * 128:(dt + 1) * 128],
                                         eye16[:], is_transpose=True)
                        nc.scalar.activation(
                            xt[:, dt, tb * 128:(tb + 1) * 128], ptr[:], AF.Copy)

                # [128,1] partial sums -> [1,1] total via ones-matmul
                def sum_tail(p3, tagsuf):
                    tps = ps_st.tile([1, 1], F32, tag="pa")
                    nc.tensor.matmul(tps[:], ones_colf[:], p3[:], start=True,
                                     stop=True)
                    tot = sml.tile([1, 1], F32, tag=f"tot{tagsuf}")
                    nc.scalar.activation(tot[:], tps[:], AF.Copy)
                    return tot

                # [1,1] total -> sneg=-max(mean,EPS) [128,1], h=s/2, -h
                def finalize_scale(tot_sb, mul, tagsuf):
                    rps = ps_st.tile([128, 1], F32, tag="pa")
                    nc.tensor.matmul(rps[:], ones_row[:], tot_sb[:], start=True,
                                     stop=True)
                    s = scl.tile([128, 1], F32, tag=f"s{tagsuf}")
                    nc.scalar.activation(s[:], rps[:], AF.Copy, scale=mul)
                    nc.vector.tensor_scalar(s[:], s[:], EPS, None, OP.max)
                    sneg = scl.tile([128, 1], F32, tag=f"sn{tagsuf}")
                    nc.vector.tensor_scalar(sneg[:], s[:], -1.0, None, OP.mult)
                    h = scl.tile([128, 1], F32, tag=f"h{tagsuf}")
                    nc.vector.tensor_scalar(h[:], s[:], 0.5, None, OP.mult)
                    nh = scl.tile([128, 1], F32, tag=f"nh{tagsuf}")
                    nc.vector.tensor_scalar(nh[:], h[:], -1.0, None, OP.mult)
                    return sneg, h, nh

                # fp16 [DIM, 256]-chunk abs-sum into an accumulator on DVE
                accs = {}

                def abs_chunk(view, ch, acc, eng=None, pool=None):
                    wt = (pool or hch).tile([128, DT, 256], F16, tag="hch")
                    (eng or nc.scalar).dma_start(
                        wt[:], view[:, :, ch * 256:(ch + 1) * 256])
                    part = sml.tile([128, DT, 2], F32, tag="partc")
                    nc.vector.tensor_reduce(
                        part[:], wt[:].rearrange("p dt (c k) -> p dt c k", k=128),
                        axis=AX.X, op=OP.add, apply_absolute_value=True)
                    p3 = sml.tile([128, 1], F32, tag="p3c")
                    nc.vector.tensor_reduce(
                        p3[:], part[:].rearrange("p dt c -> p (dt c)"),
                        axis=AX.X, op=OP.add)
                    if acc not in accs:
                        accs[acc] = sml.tile([128, 1], F32, tag=acc, name=acc)
                        nc.vector.tensor_copy(accs[acc][:], p3[:])
                    else:
                        nc.vector.tensor_tensor(accs[acc][:], accs[acc][:],
                                                p3[:], OP.add)

                # w0 scale: local fp16 abs-sum (no collective)
                for ch in range(WC):
                    abs_chunk(w0h_v, ch, "w0acc")
                sn0, h0, nh0 = finalize_scale(sum_tail(accs["w0acc"], "w0"),
                                              1.0 / (DIM * DIM), "w0")

                # ---- streamed bitlinear for trunk ----
                def bitlinear(wview, h_ap, nh_ap, rhs, consume, chunks):
                    for g in range(DT // G):
                        wt = wstream.tile([128, DT, G * 128], F32, tag="wstream")
                        eng = nc.sync if g % 2 == 0 else nc.scalar
                        eng.dma_start(wt[:], wview[:, :, g * G * 128:(g + 1) * G * 128])
                        sgn = qbuf.tile([128, DT, G * 128], F16, tag="sgn")
                        nc.vector.tensor_scalar(sgn[:], wt[:], h_ap[:], None, OP.is_gt)
                        qng = qbuf.tile([128, DT, G * 128], F16, tag="qng")
                        nc.vector.scalar_tensor_tensor(qng[:], wt[:], nh_ap[:], sgn[:],
                                                       OP.is_lt, OP.subtract)
                        for fn in chunks.get(g, []):
                            fn()
                        for j in range(G):
                            ot = g * G + j
                            pt = ps_mm.tile([128, T], F32, tag="ps_mm")
                            for dt in range(DT):
                                nc.tensor.matmul(pt[:], qng[:, dt, j * 128:(j + 1) * 128],
                                                 rhs[:, dt, :],
                                                 start=(dt == 0), stop=(dt == DT - 1))
                            consume(ot, pt)

                # ---- layer 0; head chunks + w1 chunks + AR2 interleaved ----
                h1sb = big.tile([128, DT, T], F16, tag="h1sb")

                def consume_l0(ot, pt):
                    nc.scalar.activation(h1sb[:, ot, :], pt[:], AF.Identity,
                                         bias=b0s[:, ot:ot + 1], scale=sn0[:])

                toth_g = sml.tile([1, 1], F32, tag="tothg")

                def fire_ar2():
                    toth = sum_tail(accs["hacc"], "hd")
                    nc.scalar.dma_start(ar2_in[:], toth[:])
                    nc.gpsimd.collective_compute(
                        "AllReduce", OP.add, replica_groups=groups,
                        ins=[ar2_in[:]], outs=[ar2_out[:]])
                    nc.scalar.dma_start(toth_g[:], ar2_out[:])

                # w1 fp16 chunks on scalar during L0 groups 4..7
                l0_sched = {g: [lambda c=c: abs_chunk(w1h_v, c, "w1acc")
                                for c in range(2 * (g - 4), 2 * (g - 4) + 2)]
                            for g in range(4, 8)}

                bitlinear(w0t_v, h0, nh0, xt, consume_l0, l0_sched)
                sn1, h1, nh1 = finalize_scale(sum_tail(accs["w1acc"], "w1"),
                                              1.0 / (DIM * DIM), "w1")

                # ---- layer 1 + LN stats ----
                h2sb = big.tile([128, DT, T], F16, tag="xt", name="h2sb")
                ps_s = ps_st.tile([1, T], F32, tag="ps_s")
                ps_q = ps_st.tile([1, T], F32, tag="ps_q")

                def consume_l1(ot, pt):
                    nc.scalar.activation(h2sb[:, ot, :], pt[:], AF.Identity,
                                         bias=b1s[:, ot:ot + 1], scale=sn1[:])
                    sq = evt.tile([128, T], F16, tag="evt")
                    nc.scalar.activation(sq[:], pt[:], AF.Square,
                                         bias=b1s[:, ot:ot + 1], scale=sn1[:])
                    nc.tensor.matmul(ps_s[:], ones_col[:], h2sb[:, ot, :],
                                     start=(ot == 0), stop=(ot == DT - 1))
                    nc.tensor.matmul(ps_q[:], ones_col[:], sq[:],
                                     start=(ot == 0), stop=(ot == DT - 1))

                l1_sched = {g: [lambda c=c: abs_chunk(wslh16_v, c, "hacc",
                                                      eng=nc.gpsimd, pool=hch2)
                                for c in range(2 * g, 2 * g + 2)]
                            for g in range(8)}
                bitlinear(w1t_v, h1, nh1, h1sb, consume_l1, l1_sched)
                fire_ar2()

                # head scale finalize: PE ops sit after L1 so the PE FIFO
                # doesn't stall on AR2
                snh, hh, nhh = finalize_scale(toth_g, 1.0 / (DIM * VOCAB), "hd")

                # ---- layernorm ----
                mu = sml.tile([1, T], F32, tag="mu")
                nc.scalar.activation(mu[:], ps_s[:], AF.Copy, scale=1.0 / DIM)
                ms = sml.tile([1, T], F32, tag="ms")
                nc.scalar.activation(ms[:], ps_q[:], AF.Copy, scale=1.0 / DIM)
                var = sml.tile([1, T], F32, tag="var")
                nc.vector.tensor_tensor(var[:], mu[:], mu[:], OP.mult)
                nc.vector.tensor_tensor(var[:], ms[:], var[:], OP.subtract)
                sd = sml.tile([1, T], F32, tag="sd")
                nc.scalar.activation(sd[:], var[:], AF.Sqrt, bias=eps1[:])
                rstd = sml.tile([1, T], F32, tag="rstd")
                nc.vector.reciprocal(rstd[:], sd[:])
                negmur = sml.tile([1, T], F32, tag="r0")
                nc.vector.tensor_tensor(negmur[:], mu[:], rstd[:], OP.mult)
                nc.vector.tensor_scalar(negmur[:], negmur[:], -1.0, None, OP.mult)
                # broadcast to [128, T] via ones-matmul
                pa = ps_st.tile([128, T], F32, tag="pa")
                nc.tensor.matmul(pa[:], ones_row[:], rstd[:], start=True, stop=True)
                a_b = big.tile([128, T], F32)
                nc.scalar.activation(a_b[:], pa[:], AF.Copy)
                pb = ps_st.tile([128, T], F32, tag="pa")
                nc.tensor.matmul(pb[:], ones_row[:], negmur[:], start=True, stop=True)
                b_b = big.tile([128, T], F32)
                nc.scalar.activation(b_b[:], pb[:], AF.Copy)

                # apply LN -> fp16 into per-slice tiles (contiguous AG inputs)
                xs = [big.tile([128, DT, tb], F16, tag=f"xs{i}", name=f"xs{i}")
                      for i, tb in enumerate(TBS)]
                for dt in range(DT):
                    t1 = evt.tile([128, T], F32, tag="evtf")
                    nc.vector.tensor_tensor(t1[:], h2sb[:, dt, :], a_b[:], OP.mult)
                    nc.vector.tensor_tensor(t1[:], t1[:], b_b[:], OP.add)
                    off = 0
                    for i, tb in enumerate(TBS):
                        nc.scalar.activation(xs[i][:, dt, :], t1[:, off:off + tb],
                                             AF.Identity,
                                             bias=bets[:, dt:dt + 1],
                                             scale=gams[:, dt:dt + 1])
                        off += tb

                for i, tb in enumerate(TBS):
                    nc.sync.dma_start(
                        ag_in_q[i][:].rearrange("p (dt t) -> p dt t", t=tb),
                        xs[i][:])

            # ---- head phase: token-slice-major ----
            with contextlib.ExitStack() as head_ctx:
                acts_p = head_ctx.enter_context(tc.tile_pool(name="acts", bufs=1))
                hws = head_ctx.enter_context(tc.tile_pool(name="hws", bufs=4))
                hqb = head_ctx.enter_context(tc.tile_pool(name="hqb", bufs=2))
                osb = head_ctx.enter_context(tc.tile_pool(name="osb", bufs=2))

                # collectives + acts loads interleaved on the gpsimd queue:
                # acts_b[i] is ready right after AG i, and nothing later on
                # the gpsimd queue is needed early.
                acts_b = []
                for i, tb in enumerate(TBS):
                    ab = acts_p.tile([128, DT, NCORES, tb], F16, tag=f"actb{i}",
                                     name=f"actb{i}")
                    nc.gpsimd.collective_compute(
                        "AllGather", OP.bypass, replica_groups=groups,
                        ins=[ag_in_q[i][:]], outs=[ag_out_q[i][:]])
                    for r in range(NCORES):
                        nc.gpsimd.dma_start(
                            ab[:, :, r, :], ag_out_q[i][:][r].rearrange(
                                "p (dt t) -> p dt t", t=tb))
                    acts_b.append(ab)

                def quant_head(g):
                    wt = hws.tile([128, DT, 128], F32, tag="hws")
                    eng = nc.sync if g % 2 == 0 else nc.scalar
                    eng.dma_start(wt[:], wslh_v[:, :, g * 128:(g + 1) * 128])
                    sgn = hqb.tile([128, DT, 128], F16, tag="sgn")
                    nc.vector.tensor_scalar(sgn[:], wt[:], hh[:], None, OP.is_gt)
                    qng = hqb.tile([128, DT, 128], F16, tag="qng")
                    nc.vector.scalar_tensor_tensor(qng[:], wt[:], nhh[:], sgn[:],
                                                   OP.is_lt, OP.subtract)
                    return qng

                def head_mm(ps_h, b, g, qng):
                    # NMM[b] 512-col matmuls per dt sharing one lhsT load
                    n = NMM[b]
                    rv = acts_b[b][:].rearrange("p dt r t -> p dt (r t)")
                    pts = [ps_h.tile([128, T], F32, tag=f"ps{b}_{k}",
                                     name=f"ps{b}_{k}") for k in range(n)]
                    for dt in range(DT):
                        for k in range(n):
                            nc.tensor.matmul(pts[k][:], qng[:, dt, :],
                                             rv[:, dt, k * T:(k + 1) * T],
                                             start=(dt == 0), stop=(dt == DT - 1))
                    o = osb.tile([128, NCORES * TBS[b]], F16, tag=f"osb{b}")
                    for k in range(n):
                        nc.scalar.activation(o[:, k * T:(k + 1) * T], pts[k][:],
                                             AF.Identity, bias=hbs[:, g:g + 1],
                                             scale=snh[:])
                    nc.scalar.dma_start(
                        out_b[b].ap()[g * 128:(g + 1) * 128, :], o[:])

                # pass 1: small slice (starts as soon as AG 0 lands)
                with tc.tile_pool(name="ps_h1", bufs=3, space="PSUM") as ps_h1:
                    for g in range(HT):
                        head_mm(ps_h1, 0, g, quant_head(g))

                # pass 2: big slice
                with tc.tile_pool(name="ps_h2", bufs=1, space="PSUM") as ps_h2:
                    for g in range(HT):
                        head_mm(ps_h2, 1, g, quant_head(g))

    nc.compile()
    return nc


_BUILD_CACHE = {}


def _get_nc(cfg: Cfg):
    key = (cfg.G,)
    if key not in _BUILD_CACHE:
        _BUILD_CACHE[key] = build(cfg)
    return _BUILD_CACHE[key]


def make_in_maps(cfg: Cfg, x, emb, w0, b0, w1, b1, ln_gamma, ln_beta, head_w, head_b):
    """Host-side sharding/layout prep. Returns list of per-core input dicts."""
    embh = np.asarray(emb, np.float32).astype(np.float16)
    w0t = np.ascontiguousarray(np.asarray(w0, np.float32).T)
    w1t = np.ascontiguousarray(np.asarray(w1, np.float32).T)
    hwt = np.ascontiguousarray(np.asarray(head_w, np.float32).T)  # [D, V]

    def rearr(v, n):
        return np.ascontiguousarray(np.asarray(v, np.float32).reshape(n, 128).T)

    b0r = rearr(b0, DT)
    b1r = rearr(b1, DT)
    gamr = rearr(ln_gamma, DT)
    betr = rearr(ln_beta, DT)
    hb = np.asarray(head_b, np.float32)

    ids = np.asarray(x).reshape(-1).astype(np.int16)
    assert ids.size == NCORES * T
    in_maps = []
    for c in range(NCORES):
        # indices wrapped into 16 partitions, replicated across the 8 Q7 stripes
        idx_arr = np.tile(ids[c * T:(c + 1) * T].reshape(T // 16, 16).T, (8, 1))
        lo, cnt = _START[c] * 128, _CNT[c] * 128
        wslh = np.zeros((DIM, HT * 128), np.float32)
        wslh[:, :cnt] = hwt[:, lo:lo + cnt]
        hbr = np.zeros((HT * 128,), np.float32)
        hbr[:cnt] = hb[lo:lo + cnt]
        in_maps.append(dict(
            idx=idx_arr, embh=embh, w0t=w0t, w1t=w1t,
            w0h=w0t.astype(np.float16), w1h=w1t.astype(np.float16),
            wslh=wslh, wslh16=wslh.astype(np.float16),
            b0r=b0r, b1r=b1r, gamr=gamr, betr=betr,
            hbr=rearr(hbr, HT), eye16=np.eye(128, dtype=np.float16)))
    return in_maps


def _run(cfg: Cfg, inputs, trace=False):
    nc = _get_nc(cfg)
    in_maps = make_in_maps(cfg, **inputs)
    res = run_bass_kernel_spmd(nc, in_maps, core_ids=list(range(NCORES)),
                               trace=trace)
    full = np.empty((TF, VOCAB), np.float32)
    offs = np.concatenate([[0], np.cumsum(TBS)[:-1]])
    for c in range(NCORES):
        lo, cnt = _START[c] * 128, _CNT[c] * 128
        for i, tb in enumerate(TBS):
            # out_b[i]: rows = o, cols = (rank, slice-token)
            o = res.results[c][f"out{i}"].reshape(HT * 128, NCORES, tb)
            for r in range(NCORES):
                t0 = r * T + offs[i]
                full[t0:t0 + tb, lo:lo + cnt] = o[:cnt, r, :].T
    return full, res


def kernel(**inputs) -> np.ndarray:
    cfg = Cfg()
    full, _ = _run(cfg, inputs)
    return full.reshape(BATCH, SEQ, VOCAB)
